# revision 4
# baseline (speedup 1.0000x reference)
"""SSD decode + greedy NMS (DecodeSSDPredictions) on 8 Trainium2 NeuronCores.

Data-parallel: 32 batch items sharded 4-per-core across 8 cores. Per item:
  - stream y_pred[24564, 93] into SBUF; per-box class max over classes 1..80
    (softmax rows: at most one class can be >= 0.5, and the "argmax==0"
    degenerate case is impossible unless two classes are exactly 0.5),
  - SSD box decode (variances * offsets, exp on ScalarE, corners scaled by
    512 folded as exact power-of-two multiplies); area pre-scaled by the IoU
    threshold 0.35 so suppression is `1.35*inter > 0.35*(area_a+area_b)`,
  - greedy NMS, only the first NUM_PRED=10 iterations (kept-score sequence is
    non-increasing, so top_k(100-iter kept, 10) == first 10 selections).
    Cross-partition argmax via gpsimd partition_all_reduce with exact
    flat-index tie-breaking. The selected box's fields come from an indirect
    DMA gather of its raw y row + an exact scalar re-decode (bit-identical op
    sequence), keeping the wide [128,192] work to ~10 vector ops/iteration.
    Class id of the winner computed inline from the gathered row.
"""

import sys

import numpy as np

for _p in ("/opt/trn_rl_repo", "/root/.axon_site/_ro/trn_rl_repo"):
    if _p not in sys.path:
        sys.path.insert(0, _p)

import concourse.bacc as bacc
import concourse.bass as bass
import concourse.bass_isa as bass_isa
import concourse.mybir as mybir
from concourse.bass_types import AP
from concourse.bass_utils import run_bass_kernel_spmd
from concourse.tile import TileContext

F32 = mybir.dt.float32
ALU = mybir.AluOpType
ACTF = mybir.ActivationFunctionType
AX = mybir.AxisListType
RED = bass_isa.ReduceOp

B = 32
N = 24564
NC_CLS = 81
NCORES = 8
ITEMS = B // NCORES          # 4 items per core
P = 128
TCOL = 192                   # 128*192 = 24576 >= N, p-major: box n -> (n//192, n%192)
NPAD = P * TCOL              # host pads each item to 24576 box rows (pad rows all-zero)
TMEGA = 96                   # columns per streamed mega-tile (2 per item)
ROW = 93                     # floats per box row
NSEL = 10                    # output predictions per item
CONF = 0.5
IOU_T = 0.35
IMG = 512.0
NEG = -1.0e30                # dead-score sentinel (reference uses -inf)
IOTAR_BASE = 30000.0         # reversed-index key base; > N so key stays positive

_CACHE = {}


def _host_consts() -> np.ndarray:
    flat = (np.arange(P, dtype=np.float32)[:, None] * TCOL
            + np.arange(TCOL, dtype=np.float32)[None, :])
    return IOTAR_BASE - flat                       # [128,192] reversed key (positive)


def _build():
    nc = bacc.Bacc(None, target_bir_lowering=False)
    y = nc.dram_tensor("y", [ITEMS * NPAD * ROW], F32, kind="ExternalInput")
    cst = nc.dram_tensor("cst", [P, TCOL], F32, kind="ExternalInput")
    out = nc.dram_tensor("out", [ITEMS * NSEL * 6], F32, kind="ExternalOutput")

    with TileContext(nc) as tc:
        with (
            tc.tile_pool(name="cpool", bufs=1) as cpool,
            tc.tile_pool(name="xpool", bufs=2) as xpool,
            tc.tile_pool(name="apool", bufs=1) as apool,
            tc.tile_pool(name="spool", bufs=3) as spool,
            tc.tile_pool(name="npool", bufs=6) as npool,
        ):
            cstT = cpool.tile([P, TCOL], F32)
            nc.sync.dma_start(out=cstT, in_=cst[:, :])
            iotaR = cstT[:, :]

            # ---- per-item persistent arrays ----
            scoresA, x1A, y1A, x2A, y2A, areaA, krowA = [], [], [], [], [], [], []
            for i in range(ITEMS):
                scoresA.append(apool.tile([P, TCOL], F32, name=f"scores{i}", tag=f"scores{i}"))
                x1A.append(apool.tile([P, TCOL], F32, name=f"x1_{i}", tag=f"x1_{i}"))
                y1A.append(apool.tile([P, TCOL], F32, name=f"y1_{i}", tag=f"y1_{i}"))
                x2A.append(apool.tile([P, TCOL], F32, name=f"x2_{i}", tag=f"x2_{i}"))
                y2A.append(apool.tile([P, TCOL], F32, name=f"y2_{i}", tag=f"y2_{i}"))
                areaA.append(apool.tile([P, TCOL], F32, name=f"area{i}", tag=f"area{i}"))
                # per-selection record, 8 cols per j:
                # (score, x1, y1, x2, y2, pad, idx, class_id)
                krowA.append(apool.tile([1, NSEL * 8], F32, name=f"krow{i}", tag=f"krow{i}"))

            # ================= streaming: class max + decode =================
            for i in range(ITEMS):
                for mega in range(2):
                    t0 = mega * TMEGA
                    X = xpool.tile([P, TMEGA * ROW], F32, name="X", tag="X")
                    base = i * NPAD * ROW + t0 * ROW
                    src = AP(y, base, [[TCOL * ROW, P], [1, TMEGA * ROW]])
                    nc.sync.dma_start(out=X, in_=src)

                    X3 = X.rearrange("p (t c) -> p t c", c=ROW)
                    sl = slice(t0, t0 + TMEGA)

                    # class max over classes 1..80 (class 0 can never win validly)
                    S = spool.tile([P, TMEGA], F32, name="S", tag="S")
                    nc.vector.reduce_max(out=S, in_=X3[:, :, 1:NC_CLS], axis=AX.X)
                    minv = spool.tile([P, TMEGA], F32, name="minv", tag="minv")
                    nc.vector.tensor_scalar(minv, S, CONF, None, op0=ALU.is_lt)
                    # scores0 = S (valid) / ~NEG (invalid):  S + minv*NEG
                    nc.vector.scalar_tensor_tensor(
                        scoresA[i][:, sl], minv, NEG, S, op0=ALU.mult, op1=ALU.add)

                    o_cx, o_cy = X3[:, :, 81], X3[:, :, 82]
                    o_w, o_h = X3[:, :, 83], X3[:, :, 84]
                    a_cx, a_cy = X3[:, :, 85], X3[:, :, 86]
                    a_w, a_h = X3[:, :, 87], X3[:, :, 88]
                    v0, v1 = X3[:, :, 89], X3[:, :, 90]
                    v2, v3 = X3[:, :, 91], X3[:, :, 92]

                    tcx = spool.tile([P, TMEGA], F32, name="tcx", tag="tcx")
                    nc.gpsimd.tensor_tensor(tcx, o_cx, v0, op=ALU.mult)
                    nc.gpsimd.tensor_tensor(tcx, tcx, a_w, op=ALU.mult)
                    nc.gpsimd.tensor_tensor(tcx, tcx, a_cx, op=ALU.add)   # cx
                    tcy = spool.tile([P, TMEGA], F32, name="tcy", tag="tcy")
                    nc.gpsimd.tensor_tensor(tcy, o_cy, v1, op=ALU.mult)
                    nc.gpsimd.tensor_tensor(tcy, tcy, a_h, op=ALU.mult)
                    nc.gpsimd.tensor_tensor(tcy, tcy, a_cy, op=ALU.add)   # cy

                    tw = spool.tile([P, TMEGA], F32, name="tw", tag="tw")
                    nc.vector.tensor_tensor(tw, o_w, v2, op=ALU.mult)
                    ew = spool.tile([P, TMEGA], F32, name="ew", tag="ew")
                    nc.scalar.activation(ew, tw, ACTF.Exp)
                    nc.vector.tensor_tensor(ew, ew, a_w, op=ALU.mult)     # w
                    th = spool.tile([P, TMEGA], F32, name="th", tag="th")
                    nc.vector.tensor_tensor(th, o_h, v3, op=ALU.mult)
                    eh = spool.tile([P, TMEGA], F32, name="eh", tag="eh")
                    nc.scalar.activation(eh, th, ACTF.Exp)
                    nc.vector.tensor_tensor(eh, eh, a_h, op=ALU.mult)     # h

                    # corners: (cx +- 0.5w)*512 == cx*512 +- w*256 exactly (2^k scaling)
                    nc.vector.tensor_scalar(tcx, tcx, IMG, None, op0=ALU.mult)  # cx*512
                    nc.vector.tensor_scalar(tcy, tcy, IMG, None, op0=ALU.mult)  # cy*512
                    nc.vector.scalar_tensor_tensor(
                        x1A[i][:, sl], ew, -IMG / 2, tcx, op0=ALU.mult, op1=ALU.add)
                    nc.vector.scalar_tensor_tensor(
                        x2A[i][:, sl], ew, IMG / 2, tcx, op0=ALU.mult, op1=ALU.add)
                    nc.vector.scalar_tensor_tensor(
                        y1A[i][:, sl], eh, -IMG / 2, tcy, op0=ALU.mult, op1=ALU.add)
                    nc.vector.scalar_tensor_tensor(
                        y2A[i][:, sl], eh, IMG / 2, tcy, op0=ALU.mult, op1=ALU.add)

                    dw = spool.tile([P, TMEGA], F32, name="dw", tag="dw")
                    nc.gpsimd.tensor_tensor(dw, x2A[i][:, sl], x1A[i][:, sl], op=ALU.subtract)
                    dh = spool.tile([P, TMEGA], F32, name="dh", tag="dh")
                    nc.gpsimd.tensor_tensor(dh, y2A[i][:, sl], y1A[i][:, sl], op=ALU.subtract)
                    # areaT = (dw*0.35)*dh  (IoU threshold folded into the area)
                    nc.vector.scalar_tensor_tensor(
                        areaA[i][:, sl], dw, IOU_T, dh, op0=ALU.mult, op1=ALU.mult)

            # ================= greedy NMS: 10 iterations per item =================
            # emit iteration j for all items back-to-back so the four
            # independent per-item dependency chains interleave on the engines
            yrows = AP(y, 0, [[ROW, ITEMS * NPAD], [1, ROW]])
            for j in range(NSEL):
                for i in range(ITEMS):
                    sc = scoresA[i]
                    kv = krowA[i]
                    m = npool.tile([P, 1], F32, name="m", tag="m")
                    nc.vector.reduce_max(out=m, in_=sc, axis=AX.X)
                    gm = npool.tile([P, 1], F32, name="gm", tag="gm")
                    nc.gpsimd.partition_all_reduce(gm, m, channels=P, reduce_op=RED.max)

                    # tie-break by smallest flat index: key = (score==gm) * (BASE-flat)
                    mask = npool.tile([P, TCOL], F32, name="mask", tag="mask")
                    nc.vector.tensor_scalar(mask, sc, gm[:, 0:1], None, op0=ALU.is_equal)
                    idxm = npool.tile([P, TCOL], F32, name="idxm", tag="idxm")
                    nc.gpsimd.tensor_tensor(idxm, mask, iotaR, op=ALU.mult)
                    pm = npool.tile([P, 1], F32, name="pm", tag="pm")
                    nc.vector.reduce_max(out=pm, in_=idxm, axis=AX.X)
                    gpm = npool.tile([P, 1], F32, name="gpm", tag="gpm")
                    nc.gpsimd.partition_all_reduce(gpm, pm, channels=P, reduce_op=RED.max)

                    # record score + local flat idx (idx = BASE - gpm, exact in f32)
                    nc.scalar.copy(kv[0:1, 8 * j:8 * j + 1], gm[0:1, 0:1])
                    nc.vector.tensor_scalar(
                        kv[0:1, 8 * j + 6:8 * j + 7], gpm[0:1, 0:1],
                        -1.0, IOTAR_BASE, op0=ALU.mult, op1=ALU.add)
                    # global row idx for the gather (+ item offset), as int32.
                    # gpm is partition-replicated, so a [2,1] slice holds the
                    # same index twice (1-element indirect DMA is unsupported).
                    idxf = npool.tile([2, 1], F32, name="idxf", tag="idxf")
                    nc.vector.tensor_scalar(
                        idxf, gpm[0:2, 0:1], -1.0, IOTAR_BASE + float(i * NPAD),
                        op0=ALU.mult, op1=ALU.add)
                    idxi = npool.tile([2, 1], mybir.dt.int32, name="idxi", tag="idxi")
                    nc.vector.tensor_copy(idxi, idxf)

                    # gather the selected box's raw row and re-decode it exactly
                    # (same ALU op sequence as the dense decode -> bit-identical)
                    row = npool.tile([2, ROW], F32, name="row", tag="row")
                    nc.gpsimd.indirect_dma_start(
                        out=row, out_offset=None, in_=yrows,
                        in_offset=bass.IndirectOffsetOnAxis(ap=idxi[:, 0:1], axis=0))
                    r = lambda k: row[0:1, k:k + 1]
                    sel = npool.tile([1, 8], F32, name="sel", tag="sel")
                    scx = npool.tile([1, 1], F32, name="scx", tag="scx")
                    nc.vector.tensor_tensor(scx, r(81), r(89), op=ALU.mult)
                    nc.vector.tensor_tensor(scx, scx, r(87), op=ALU.mult)
                    nc.vector.tensor_tensor(scx, scx, r(85), op=ALU.add)
                    nc.vector.tensor_scalar(scx, scx, IMG, None, op0=ALU.mult)
                    scy = npool.tile([1, 1], F32, name="scy", tag="scy")
                    nc.vector.tensor_tensor(scy, r(82), r(90), op=ALU.mult)
                    nc.vector.tensor_tensor(scy, scy, r(88), op=ALU.mult)
                    nc.vector.tensor_tensor(scy, scy, r(86), op=ALU.add)
                    nc.vector.tensor_scalar(scy, scy, IMG, None, op0=ALU.mult)
                    stw = npool.tile([1, 1], F32, name="stw", tag="stw")
                    nc.vector.tensor_tensor(stw, r(83), r(91), op=ALU.mult)
                    sew = npool.tile([1, 1], F32, name="sew", tag="sew")
                    nc.scalar.activation(sew, stw, ACTF.Exp)
                    nc.vector.tensor_tensor(sew, sew, r(87), op=ALU.mult)
                    sth = npool.tile([1, 1], F32, name="sth", tag="sth")
                    nc.vector.tensor_tensor(sth, r(84), r(92), op=ALU.mult)
                    seh = npool.tile([1, 1], F32, name="seh", tag="seh")
                    nc.scalar.activation(seh, sth, ACTF.Exp)
                    nc.vector.tensor_tensor(seh, seh, r(88), op=ALU.mult)
                    # sel = (x1, y1, x2, y2, areaT)
                    nc.vector.scalar_tensor_tensor(
                        sel[0:1, 0:1], sew, -IMG / 2, scx, op0=ALU.mult, op1=ALU.add)
                    nc.vector.scalar_tensor_tensor(
                        sel[0:1, 1:2], seh, -IMG / 2, scy, op0=ALU.mult, op1=ALU.add)
                    nc.vector.scalar_tensor_tensor(
                        sel[0:1, 2:3], sew, IMG / 2, scx, op0=ALU.mult, op1=ALU.add)
                    nc.vector.scalar_tensor_tensor(
                        sel[0:1, 3:4], seh, IMG / 2, scy, op0=ALU.mult, op1=ALU.add)
                    sdw = npool.tile([1, 1], F32, name="sdw", tag="sdw")
                    nc.vector.tensor_tensor(sdw, sel[0:1, 2:3], sel[0:1, 0:1], op=ALU.subtract)
                    sdh = npool.tile([1, 1], F32, name="sdh", tag="sdh")
                    nc.vector.tensor_tensor(sdh, sel[0:1, 3:4], sel[0:1, 1:2], op=ALU.subtract)
                    nc.vector.scalar_tensor_tensor(
                        sel[0:1, 4:5], sdw, IOU_T, sdh, op0=ALU.mult, op1=ALU.mult)
                    nc.scalar.copy(kv[0:1, 8 * j + 1:8 * j + 5], sel[0:1, 0:4])

                    # class id of the winner from the gathered row
                    cm8 = npool.tile([1, 8], F32, name="cm8", tag="cm8")
                    nc.vector.max(out=cm8, in_=row[0:1, 0:NC_CLS])
                    ci8 = npool.tile([1, 8], mybir.dt.uint32, name="ci8", tag="ci8")
                    nc.vector.max_index(ci8, cm8, row[0:1, 0:NC_CLS])
                    nc.vector.tensor_copy(kv[0:1, 8 * j + 7:8 * j + 8], ci8[:, 0:1])

                    # broadcast (x1,y1,x2,y2,areaT) to all partitions
                    selb = npool.tile([P, 8], F32, name="selb", tag="selb")
                    nc.gpsimd.partition_broadcast(selb, sel[0:1, :], channels=P)

                    # suppression: sc += NEG iff (1.35*inter > areaT+sareaT) & ok
                    a1 = npool.tile([P, TCOL], F32, name="a1", tag="a1")
                    nc.vector.tensor_scalar(a1, x1A[i], selb[:, 0:1], None, op0=ALU.max)
                    Bx = npool.tile([P, TCOL], F32, name="Bx", tag="Bx")
                    nc.vector.scalar_tensor_tensor(
                        Bx, x2A[i], selb[:, 2:3], a1, op0=ALU.min, op1=ALU.subtract)
                    iw = npool.tile([P, TCOL], F32, name="iw", tag="iw")
                    nc.scalar.activation(iw, Bx, ACTF.Relu)
                    c1 = npool.tile([P, TCOL], F32, name="c1", tag="c1")
                    nc.vector.tensor_scalar(c1, y1A[i], selb[:, 1:2], None, op0=ALU.max)
                    Dy = npool.tile([P, TCOL], F32, name="Dy", tag="Dy")
                    nc.vector.scalar_tensor_tensor(
                        Dy, y2A[i], selb[:, 3:4], c1, op0=ALU.min, op1=ALU.subtract)
                    ih = npool.tile([P, TCOL], F32, name="ih", tag="ih")
                    nc.scalar.activation(ih, Dy, ACTF.Relu)
                    inter = npool.tile([P, TCOL], F32, name="inter", tag="inter")
                    nc.gpsimd.tensor_tensor(inter, iw, ih, op=ALU.mult)
                    t = npool.tile([P, TCOL], F32, name="t", tag="t")
                    nc.vector.tensor_scalar(t, areaA[i], selb[:, 4:5], None, op0=ALU.add)
                    mk = npool.tile([P, TCOL], F32, name="mk", tag="mk")
                    nc.vector.scalar_tensor_tensor(
                        mk, inter, 1.0 + IOU_T, t, op0=ALU.mult, op1=ALU.is_gt)
                    negok = npool.tile([P, 1], F32, name="negok", tag="negok")
                    nc.vector.tensor_scalar(negok, gm, CONF, NEG, op0=ALU.is_ge, op1=ALU.mult)
                    nc.vector.scalar_tensor_tensor(
                        sc, mk, negok[:, 0:1], sc, op0=ALU.mult, op1=ALU.add)

            # ================= output assembly =================
            stage = cpool.tile([1, ITEMS * NSEL * 6], F32)
            for i in range(ITEMS):
                kv = krowA[i].rearrange("a (j f) -> a j f", f=8)
                vrow = npool.tile([1, NSEL], F32, name="vrow", tag="vrow")
                nc.vector.tensor_scalar(vrow, kv[:, :, 0], CONF, None, op0=ALU.is_ge)
                sv = stage.rearrange("a (j f) -> a j f", f=6)
                ssl = sv[:, i * NSEL:(i + 1) * NSEL, :]
                nc.vector.tensor_tensor(ssl[:, :, 0], kv[:, :, 7], vrow, op=ALU.mult)
                nc.vector.tensor_tensor(ssl[:, :, 1], kv[:, :, 0], vrow, op=ALU.mult)
                nc.vector.tensor_tensor(ssl[:, :, 2], kv[:, :, 1], vrow, op=ALU.mult)
                nc.vector.tensor_tensor(ssl[:, :, 3], kv[:, :, 2], vrow, op=ALU.mult)
                nc.vector.tensor_tensor(ssl[:, :, 4], kv[:, :, 3], vrow, op=ALU.mult)
                nc.vector.tensor_tensor(ssl[:, :, 5], kv[:, :, 4], vrow, op=ALU.mult)

            nc.sync.dma_start(out=out[:], in_=stage[0:1, :])
    nc.finalize()
    return nc


def _in_maps(y_pred: np.ndarray) -> list:
    ypad = np.zeros((B, NPAD, ROW), np.float32)
    ypad[:, :N, :] = y_pred
    consts = _host_consts()
    in_maps = []
    for c in range(NCORES):
        shard = np.ascontiguousarray(ypad[c * ITEMS:(c + 1) * ITEMS]).reshape(-1)
        in_maps.append({"y": shard, "cst": consts})
    return in_maps


def kernel(y_pred: np.ndarray) -> np.ndarray:
    assert y_pred.shape == (B, N, ROW) and y_pred.dtype == np.float32
    if "nc" not in _CACHE:
        _CACHE["nc"] = _build()
    nc = _CACHE["nc"]

    res = run_bass_kernel_spmd(nc, _in_maps(y_pred), core_ids=list(range(NCORES)))
    outs = [res.results[c]["out"].reshape(ITEMS, NSEL, 6) for c in range(NCORES)]
    return np.concatenate(outs, axis=0)


if __name__ == "__main__":
    rng = np.random.default_rng(0)
    yp = rng.standard_normal((B, N, ROW), dtype=np.float32).astype(np.float32)
    print(kernel(y_pred=yp).shape)
